# revision 6
# baseline (speedup 1.0000x reference)
"""Trainium2 Bass kernel for nn_BatchProgramClassifier (gnn_message_passing).

Data-parallel over batch B=128 across 8 NeuronCores (16 programs/core).

Sharding/layout choice: the token-id lookup is folded into the input layout
host-side (each core receives its embedding rows pre-arranged in statement
order, same bytes as a replicated-table shard); all model FLOPs run on
device:

  P1: per-chunk tree aggregation tmp = G^T A (ancestor-closure matrix from
      `parents`, host index preprocessing), projection hT = Wc tmp + b s^T
      (s = subtree sizes, rank-1 accumulate on PE), windowed max + relu ->
      statement encodings.
  P2: GRU input projections x@Wih^T as wide matmuls.
  P3: 128-step bidirectional GRU scan in [H, B] layout (both directions
      packed in shared ops), running max-pool, linear classifier.
"""

import sys
import numpy as np

sys.path.insert(0, "/opt/trn_rl_repo")

import concourse.bass as bass
import concourse.tile as tile
from concourse import mybir
from concourse.bass_utils import run_bass_kernel_spmd
from concourse.masks import make_identity
from concourse.library_overlay import lower_extended_insts
from concourse.vector_clock import ScopedClock
import ml_dtypes

F32 = mybir.dt.float32
BF16 = mybir.dt.bfloat16
I32 = mybir.dt.int32
AX = mybir.AxisListType
OP = mybir.AluOpType
ACTF = mybir.ActivationFunctionType

# problem dims (hardcoded per contract)
B, L, N = 128, 128, 16
V, E, D, H, C = 30000, 128, 128, 100, 104
M = 8                 # cores
BL = B // M           # 16 programs per core
T = BL * L            # 2048 statements per core
NCH = T // 8          # 256 chunks of 8 statements
NG = 8                # chunk groups
CPG = NCH // NG       # 32 chunks per group

# ---------------------------------------------------------------------------
# TileContext tail-drain patch: the walrus in this container rejects the tail
# Drain when it carries many sem waits ("Too many sync wait commands").
# Hoist the waits onto single-wait NOPs ahead of the drain.
# ---------------------------------------------------------------------------
def _patched_drain_and_barrier(self, tick_clock, wait_clock):
    probe = self.nc.sync.nop(nofuse=True)
    wait_clock.add_sem_waits(probe.ins, ScopedClock({None: tick_clock.global_clock}))
    si = probe.ins.sync_info
    if si is not None and len(si.on_wait) > 1:
        rest = list(si.on_wait[1:])
        del si.on_wait[1:]
        for w in rest:
            nop = self.nc.sync.nop(nofuse=True)
            nsi = nop.ins.sync_info
            if nsi is None:
                nop.ins.sync_info = type(si)(on_wait=[w], on_update=[])
            else:
                nsi.on_wait.append(w)
    self.nc.sync.drain()
    self.nc.all_engine_barrier()
    assert self.sems is not None
    popped = self.nc._tile_sem_poison_stack.pop()
    assert popped is self._sem_poison
    self.nc.clear_and_free_semaphores(list(self.sems.allocated().values()))
    self.nc.all_engine_barrier()


tile.TileContext._drain_and_barrier = _patched_drain_and_barrier


def _split_sync_waits(nc, max_waits=1):
    """walrus in this container allows only one sem-wait per instruction:
    hoist extra waits onto same-engine NOPs spliced immediately before."""
    for fn in nc.m.functions:
        for bb in fn.blocks:
            out = []
            for inst in bb.instructions:
                si = inst.sync_info
                if si is not None and len(si.on_wait) > max_waits:
                    extra = list(si.on_wait[max_waits:])
                    del si.on_wait[max_waits:]
                    for w in extra:
                        out.append(mybir.InstNoOp(
                            name=nc.get_next_instruction_name(),
                            engine=inst.engine,
                            sync_info=mybir.SyncInfo(on_wait=[w], on_update=[]),
                            bass_nofuse=True,
                        ))
                out.append(inst)
            bb.instructions = out


# ---------------------------------------------------------------------------
# Device kernel
# ---------------------------------------------------------------------------
def _build(ncores=M, split_waits=True, phases=('p1', 'p2', 'p3'), mock_cc=False):
    nc = bass.Bass()
    p_g = nc.declare_dram_parameter("g_rows", [128, NG * CPG * 128], BF16, isOutput=False)
    p_ablk = nc.declare_dram_parameter("a_blk", [NG * 128, CPG * 128], BF16, isOutput=False)
    p_ssz = nc.declare_dram_parameter("s_sizes", [1, NG * CPG * 128], BF16, isOutput=False)
    p_wcT = nc.declare_dram_parameter("wcT", [E, D], BF16, isOutput=False)
    p_wcb = nc.declare_dram_parameter("wcb", [1, D], BF16, isOutput=False)
    p_wihT = {d: nc.declare_dram_parameter(f"wihT_{d}", [D, 3 * H], BF16, isOutput=False)
              for d in ("f", "b")}
    p_xbias = {d: nc.declare_dram_parameter(f"xbias_{d}", [H, 3], F32, isOutput=False)
               for d in ("f", "b")}
    p_whhT = {d: nc.declare_dram_parameter(f"whhT_{d}", [H + 1, 3 * H], F32, isOutput=False)
              for d in ("f", "b")}
    p_lblT = nc.declare_dram_parameter("lblT", [H + 1, 2 * C], F32, isOutput=False)
    p_sinit = nc.declare_dram_parameter("slab_init", [H + 1, 8 * 32], F32, isOutput=False)
    p_out = nc.declare_dram_parameter("out", [BL, C], F32, isOutput=True)

    with tile.TileContext(nc) as tc:
        with tc.tile_pool(name="const", bufs=1) as const:
            wcT_sb = const.tile([E, D], BF16)
            nc.sync.dma_start(wcT_sb[:], p_wcT[:])
            wcb_sb = const.tile([1, D], BF16)
            nc.sync.dma_start(wcb_sb[:], p_wcb[:])
            ssz_sb = const.tile([1, NG * CPG * 128], BF16)
            nc.sync.dma_start(ssz_sb[:], p_ssz[:])
            whhT_sb = {}
            wihT_sb = {}
            xbias_sb = {}
            for d in ("f", "b"):
                whhT_sb[d] = const.tile([H + 1, 3 * H], F32, name=f"whhT{d}")
                nc.sync.dma_start(whhT_sb[d][:], p_whhT[d][:])
                wihT_sb[d] = const.tile([D, 3 * H], BF16, name=f"wihT{d}")
                nc.sync.dma_start(wihT_sb[d][:], p_wihT[d][:])
                xbias_sb[d] = const.tile([H, 3], F32, name=f"xbias{d}")
                nc.sync.dma_start(xbias_sb[d][:], p_xbias[d][:])
            lblT_sb = const.tile([H + 1, 2 * C], F32)
            nc.sync.dma_start(lblT_sb[:], p_lblT[:])

            enc_sb = const.tile([128, T], BF16)
            # xW slabs: [H, dir, gate, b, l] for r/z ; [H, dir, b, l] for n
            xw_rz = const.tile([H, 2 * 2 * BL * L], BF16)
            xw_n = const.tile([H, 2 * BL * L], BF16)
            identB = const.tile([128, 128], BF16)
            make_identity(nc, identB[:])

            # ---------------- P1: tree-aggregate + project + enc -----------
            with tc.tile_pool(name="p1", bufs=2) as p1, \
                 tc.tile_pool(name="p1ps", bufs=2, space="PSUM") as p1ps:
                for g in range(NG if "p1" in phases else 0):
                    g_sb = p1.tile([128, CPG * 128], BF16, tag="g")
                    nc.sync.dma_start(g_sb[:], p_g[:, g * CPG * 128:(g + 1) * CPG * 128])
                    g_v = g_sb[:].rearrange("p (c e) -> p c e", c=CPG)
                    ab_sb = p1.tile([128, CPG * 128], BF16, tag="ab")
                    nc.sync.dma_start(ab_sb[:], p_ablk[g * 128:(g + 1) * 128, :])
                    for k in range(CPG // 4):
                        hT_ps = p1ps.tile([128, 512], F32, tag="hT")
                        # subtree-size bias: hT += Wc_b (x) s  (rank-1, no dep)
                        nc.tensor.matmul(
                            out=hT_ps[:], lhsT=wcb_sb[:],
                            rhs=ssz_sb[0:1, g * CPG * 128 + k * 512:g * CPG * 128 + (k + 1) * 512],
                            start=True, stop=False)
                        for q in range(4):
                            c = k * 4 + q
                            tmp_ps = p1ps.tile([128, 128], F32, tag="tmp", bufs=4)
                            nc.tensor.matmul(
                                out=tmp_ps[:], lhsT=g_v[:, c, :],
                                rhs=ab_sb[:, c * 128:(c + 1) * 128],
                                start=True, stop=True)
                            tmp_sb = p1.tile([128, 128], BF16, tag="tmp_sb", bufs=4)
                            nc.scalar.copy(out=tmp_sb[:], in_=tmp_ps[:])
                            nc.tensor.matmul(
                                out=hT_ps[:, q * 128:(q + 1) * 128],
                                lhsT=wcT_sb[:], rhs=tmp_sb[:],
                                start=False, stop=(q == 3), skip_group_check=True)
                        blk = g * (CPG // 4) + k   # 32 statements per block
                        nc.vector.tensor_reduce(
                            out=enc_sb[:, blk * 32:(blk + 1) * 32],
                            in_=hT_ps[:].rearrange("p (s x) -> p s x", x=N),
                            axis=AX.X, op=OP.max,
                        )
            if "p1" in phases:
                nc.scalar.activation(enc_sb[:], enc_sb[:], ACTF.Relu)

            # ---------------- P2: xW = enc @ Wih^T + bias -------------------
            with tc.tile_pool(name="p2ps", bufs=2, space="PSUM") as p2ps:
                for di, d in enumerate(("f", "b")) if "p2" in phases else ():
                    for gi in range(3):
                        for tch in range(4):
                            ps = p2ps.tile([H, 512], F32, tag="xw")
                            nc.tensor.matmul(
                                out=ps[:],
                                lhsT=wihT_sb[d][:, gi * H:(gi + 1) * H],
                                rhs=enc_sb[:, tch * 512:(tch + 1) * 512],
                                start=True, stop=True,
                            )
                            if gi < 2:
                                dest = xw_rz[:].rearrange(
                                    "p (g d b l) -> p g d b l", d=2, g=2, b=BL)[
                                    :, gi, di, tch * 4:(tch + 1) * 4, :]
                            else:
                                dest = xw_n[:].rearrange(
                                    "p (d b l) -> p d b l", d=2, b=BL)[
                                    :, di, tch * 4:(tch + 1) * 4, :]
                            nc.scalar.activation(dest, ps[:], ACTF.Identity,
                                                 bias=xbias_sb[d][:, gi:gi + 1])

            # ---------------- P3: bidirectional GRU scan --------------------
            slab = const.tile([H + 1, 8 * 32], F32)       # [H+1, slot, 2*BL]
            slab_v = slab[:].rearrange("q (s b) -> q s b", s=8)
            nc.sync.dma_start(slab[:], p_sinit[:])        # zeros + ones bias row
            pool_t = const.tile([H, 32], F32)
            from dataclasses import replace as _rep
            xwrz_base = xw_rz[:]
            xwn_base = xw_n[:]

            def xwrz_step(i):
                # element (g, dir, b): fwd at l=i, bwd at l=127-i
                sd = BL * L + (L - 1) - 2 * i
                return _rep(xwrz_base, offset=xwrz_base.offset + i,
                            ap=type(xwrz_base.ap)(
                                [list(xwrz_base.ap[0]), [2 * BL * L, 2], [sd, 2], [L, BL]]))

            def xwn_step(i):
                sd = BL * L + (L - 1) - 2 * i
                return _rep(xwn_base, offset=xwn_base.offset + i,
                            ap=type(xwn_base.ap)(
                                [list(xwn_base.ap[0]), [sd, 2], [L, BL]]))

            with tc.tile_pool(name="p3", bufs=4) as p3, \
                 tc.tile_pool(name="p3ps", bufs=2, space="PSUM") as p3ps:
                for i in range(L if "p3" in phases else 0):
                    s, pv = i % 8, (i - 1) % 8
                    hf = slab_v[0:H, pv, 0:BL]
                    hb = slab_v[0:H, pv, BL:2 * BL]
                    hf_e = slab_v[:, pv, 0:BL]
                    hb_e = slab_v[:, pv, BL:2 * BL]
                    ps_rz = p3ps.tile([H, 64], F32, tag="rz", bufs=4)
                    ps_n = p3ps.tile([H, 32], F32, tag="n", bufs=3)
                    # cols: [r_f r_b | -z_f -z_b] (z gate negated host-side so
                    # one sigmoid yields r and zbar together)
                    # xW lands first (start=True, no dep on h) so PE queues it
                    # ahead of the chain; gate matmuls accumulate onto it
                    nc.tensor.matmul(out=ps_rz[:], lhsT=identB[0:H, 0:H],
                                     rhs=xwrz_step(i), start=True, stop=False,
                                     skip_group_check=True)
                    nc.tensor.matmul(out=ps_rz[:, 0:16], lhsT=whhT_sb["f"][0:H, 0:H],
                                     rhs=hf, start=False, stop=False, skip_group_check=True)
                    nc.tensor.matmul(out=ps_rz[:, 16:32], lhsT=whhT_sb["b"][0:H, 0:H],
                                     rhs=hb, start=False, stop=False, skip_group_check=True)
                    nc.tensor.matmul(out=ps_rz[:, 32:48], lhsT=whhT_sb["f"][0:H, H:2 * H],
                                     rhs=hf, start=False, stop=False, skip_group_check=True)
                    nc.tensor.matmul(out=ps_rz[:, 48:64], lhsT=whhT_sb["b"][0:H, H:2 * H],
                                     rhs=hb, start=False, stop=True, skip_group_check=True)
                    nc.tensor.matmul(out=ps_n[:, 0:16], lhsT=whhT_sb["f"][:, 2 * H:3 * H],
                                     rhs=hf_e, start=True, stop=True)
                    nc.tensor.matmul(out=ps_n[:, 16:32], lhsT=whhT_sb["b"][:, 2 * H:3 * H],
                                     rhs=hb_e, start=True, stop=True)
                    rz = p3.tile([H, 64], F32, tag="rz_sb")
                    nc.scalar.activation(rz[:], ps_rz[:], ACTF.Sigmoid)
                    # z*h' = (1-zbar)*h' (on Pool, off the ACT/DVE chain)
                    zh = p3.tile([H, 32], F32, tag="zh")
                    nc.gpsimd.tensor_tensor(out=zh[:], in0=rz[:, 32:64],
                                            in1=slab_v[0:H, pv, :], op=OP.mult)
                    nc.gpsimd.tensor_tensor(out=zh[:], in0=slab_v[0:H, pv, :],
                                            in1=zh[:], op=OP.subtract)
                    u = p3.tile([H, 32], F32, tag="u")
                    nc.vector.tensor_tensor(out=u[:], in0=rz[:, 0:32],
                                            in1=ps_n[:], op=OP.mult)
                    t2 = p3.tile([H, 32], F32, tag="t2")
                    nc.vector.tensor_tensor(out=t2[:], in0=u[:],
                                            in1=xwn_step(i), op=OP.add)
                    # tanh(t2) = 2*sigmoid(2*t2) - 1; state kept offset by +1
                    # (h' = h+1; all bias corrections folded host-side)
                    q = p3.tile([H, 32], F32, tag="q")
                    nc.scalar.activation(q[:], t2[:], ACTF.Sigmoid, scale=2.0)
                    # hnew' = 2*q*zbar + z*h'   (only 2 ops after q arrives)
                    w1 = p3.tile([H, 32], F32, tag="w1")
                    nc.vector.tensor_tensor(out=w1[:], in0=q[:], in1=rz[:, 32:64],
                                            op=OP.mult)
                    nc.vector.scalar_tensor_tensor(
                        out=slab_v[0:H, s, :], in0=w1[:], scalar=2.0,
                        in1=zh[:], op0=OP.mult, op1=OP.add)
                    if i % 8 == 7:
                        if i == 7:
                            nc.vector.tensor_reduce(
                                out=pool_t[:], in_=slab[0:H, :].rearrange("q (s b) -> q b s", s=8),
                                axis=AX.X, op=OP.max)
                        else:
                            red = p3.tile([H, 32], F32, tag="red")
                            nc.vector.tensor_reduce(
                                out=red[:], in_=slab[0:H, :].rearrange("q (s b) -> q b s", s=8),
                                axis=AX.X, op=OP.max)
                            nc.vector.tensor_tensor(out=pool_t[:], in0=pool_t[:],
                                                    in1=red[:], op=OP.max)

                # ------------ classifier ---------------------------------
                if "p3" in phases:
                    pe = p3.tile([H + 1, 32], F32, tag="pe")
                    nc.sync.dma_start(pe[H:H + 1, :], p_sinit[H:H + 1, 0:32])
                    nc.vector.tensor_copy(out=pe[0:H, :], in_=pool_t[:])
                    ps_o = p3ps.tile([BL, C], F32, tag="out", bufs=1)
                    nc.tensor.matmul(out=ps_o[:], lhsT=pe[:, 0:16], rhs=lblT_sb[:, 0:C],
                                     start=True, stop=False)
                    nc.tensor.matmul(out=ps_o[:], lhsT=pe[:, 16:32], rhs=lblT_sb[:, C:2 * C],
                                     start=False, stop=True)
                    out_sb = p3.tile([BL, C], F32, tag="out_sb")
                    nc.vector.tensor_copy(out=out_sb[:], in_=ps_o[:])
                    nc.sync.dma_start(p_out[:], out_sb[:])
    lower_extended_insts(nc)
    if split_waits:
        _split_sync_waits(nc)
    return nc


_NC_CACHE = None


def _get_nc():
    global _NC_CACHE
    if _NC_CACHE is None:
        _NC_CACHE = _build()
    return _NC_CACHE


# ---------------------------------------------------------------------------
# Host-side input prep (sharding + index/layout preprocessing only)
# ---------------------------------------------------------------------------
def _ancestor_blocks(parents_core: np.ndarray):
    """parents_core [T, N] -> (block-diagonal ancestor-closure rhs tiles,
    subtree sizes).

    A[t, p, j] = 1 iff p is an ancestor-or-self of j; the device applies the
    tree scatter-add as H^T = G_chunk^T @ A_blk on PE.
    Returns ([NG*128, CPG*128] bf16, [NG, CPG*128] bf16 subtree sizes).
    """
    par = parents_core
    A = np.zeros((T, N, N), np.float32)
    rng = np.arange(N)
    A[:, rng, rng] = 1.0
    tidx = np.arange(T)
    for j in range(1, N):
        A[:, :, j] += A[tidx, :, par[:, j]]
    s = A.sum(axis=2)                # [T, N] subtree sizes
    # rhs_blk[(s,j),(s,p)] = A[stmt, p, j]
    At = np.transpose(A, (0, 2, 1))  # [T, j, p]
    blk = np.zeros((NCH, 128, 128), np.float32)
    Ar = At.reshape(NCH, 8, N, N)
    for st in range(8):
        blk[:, st * N:(st + 1) * N, st * N:(st + 1) * N] = Ar[:, st]
    # group-pack: [NG, 128(j-row), CPG, 128(p-col)] -> [NG*128, CPG*128]
    grp = blk.reshape(NG, CPG, 128, 128).transpose(0, 2, 1, 3).reshape(NG * 128, CPG * 128)
    ssz = s.reshape(1, NG * CPG * 128)
    return grp.astype(ml_dtypes.bfloat16), ssz.astype(ml_dtypes.bfloat16)


def kernel(tokens, parents, emb, Wc_w, Wc_b,
           Wih_f, Whh_f, bih_f, bhh_f,
           Wih_b, Whh_b, bih_b, bhh_b,
           lbl_w, lbl_b):
    tokens = np.asarray(tokens)
    parents = np.asarray(parents)
    bf = ml_dtypes.bfloat16
    emb_bf = np.asarray(emb, np.float32).astype(bf)           # [V, E]

    wcT = Wc_w.T.astype(bf)                                   # [E, D]
    wcb = np.asarray(Wc_b, np.float32).astype(bf).reshape(1, D)

    def pack_dir(Wih, Whh, bih, bhh):
        wihT = np.asarray(Wih, np.float32).T.astype(bf)       # [D, 3H]
        Whh = np.asarray(Whh, np.float32)
        bih = np.asarray(bih, np.float32)
        bhh = np.asarray(bhh, np.float32)
        rs = Whh.reshape(3, H, H).sum(axis=2)   # rowsums per gate (h'=h+1 fold)
        whhT = np.zeros((H + 1, 3 * H), np.float32)
        whhT[:H, :] = Whh.T
        whhT[:H, H:2 * H] *= -1.0               # z gate negated: sigmoid(-a_z)=zbar
        whhT[H, 2 * H:3 * H] = bhh[2 * H:3 * H] - rs[2]
        wihT[:, H:2 * H] *= -1.0
        xbias = np.stack([
            bih[0:H] + bhh[0:H] - rs[0],
            -(bih[H:2 * H] + bhh[H:2 * H] - rs[1]),
            bih[2 * H:3 * H],
        ], axis=1)                                            # [H, 3]
        return wihT, whhT, xbias

    wihT_f, whhT_f, xbias_f = pack_dir(Wih_f, Whh_f, bih_f, bhh_f)
    wihT_b, whhT_b, xbias_b = pack_dir(Wih_b, Whh_b, bih_b, bhh_b)

    lblT = np.zeros((H + 1, 2 * C), np.float32)
    lblT[:H, 0:C] = np.asarray(lbl_w, np.float32)[:, 0:H].T
    lblT[H, 0:C] = (np.asarray(lbl_b, np.float32)
                    - np.asarray(lbl_w, np.float32).sum(axis=1))
    lblT[:H, C:2 * C] = np.asarray(lbl_w, np.float32)[:, H:2 * H].T

    slab_init = np.ones((H + 1, 8 * 32), np.float32)  # h' = h+1 -> h0' = 1

    in_maps = []
    for i in range(M):
        bs = slice(i * BL, (i + 1) * BL)
        tok = tokens[bs].reshape(-1)                          # [T*N] b-major
        par = parents[bs].reshape(T, N)
        a_blk, ssz = _ancestor_blocks(par)
        # g_rows[p, ((g c) e)] = emb_bf[tok[g*4096 + c*128 + p]]
        idx = tok.reshape(NG * CPG, 128).T                    # [128, NG*CPG]
        g_rows = np.ascontiguousarray(emb_bf[idx]).reshape(128, NG * CPG * 128)
        in_maps.append(dict(
            g_rows=g_rows,
            a_blk=a_blk, s_sizes=ssz,
            wcT=wcT, wcb=wcb,
            wihT_f=wihT_f, wihT_b=wihT_b, slab_init=slab_init,
            xbias_f=xbias_f, xbias_b=xbias_b,
            whhT_f=whhT_f, whhT_b=whhT_b,
            lblT=lblT,
        ))

    nc = _get_nc()
    res = run_bass_kernel_spmd(nc, in_maps, core_ids=list(range(M)))
    return np.concatenate([res.results[i]["out"] for i in range(M)], axis=0)


# revision 13
# speedup vs baseline: 1.1380x; 1.1380x over previous
"""Trainium2 Bass kernel for nn_BatchProgramClassifier (gnn_message_passing).

Data-parallel over batch B=128 across 8 NeuronCores (16 programs/core).

Sharding/layout choice: the token-id lookup is folded into the input layout
host-side (each core receives its embedding rows pre-arranged in statement
order, same bytes as a replicated-table shard); all model FLOPs run on
device:

  P1: per-chunk tree aggregation tmp = G^T A (ancestor-closure matrix from
      `parents`, host index preprocessing), projection hT = Wc tmp + b s^T
      (s = subtree sizes, rank-1 accumulate on PE), windowed max + relu ->
      statement encodings.
  P2: GRU input projections x@Wih^T as wide matmuls.
  P3: 128-step bidirectional GRU scan in [H, B] layout (both directions
      packed in shared ops), running max-pool, linear classifier.
"""

import sys
import numpy as np

sys.path.insert(0, "/opt/trn_rl_repo")

import concourse.bass as bass
import concourse.tile as tile
from concourse import mybir
from concourse.bass_utils import run_bass_kernel_spmd
from concourse.masks import make_identity
from concourse.library_overlay import lower_extended_insts
from concourse.vector_clock import ScopedClock
import ml_dtypes

F32 = mybir.dt.float32
BF16 = mybir.dt.bfloat16
I32 = mybir.dt.int32
AX = mybir.AxisListType
OP = mybir.AluOpType
ACTF = mybir.ActivationFunctionType

# problem dims (hardcoded per contract)
B, L, N = 128, 128, 16
V, E, D, H, C = 30000, 128, 128, 100, 104
M = 8                 # cores
BL = B // M           # 16 programs per core
T = BL * L            # 2048 statements per core
NCH = T // 8          # 256 chunks of 8 statements
NG = 8                # chunk groups
CPG = NCH // NG       # 32 chunks per group

# ---------------------------------------------------------------------------
# TileContext tail-drain patch: the walrus in this container rejects the tail
# Drain when it carries many sem waits ("Too many sync wait commands").
# Hoist the waits onto single-wait NOPs ahead of the drain.
# ---------------------------------------------------------------------------
def _patched_drain_and_barrier(self, tick_clock, wait_clock):
    probe = self.nc.sync.nop(nofuse=True)
    wait_clock.add_sem_waits(probe.ins, ScopedClock({None: tick_clock.global_clock}))
    si = probe.ins.sync_info
    if si is not None and len(si.on_wait) > 1:
        rest = list(si.on_wait[1:])
        del si.on_wait[1:]
        for w in rest:
            nop = self.nc.sync.nop(nofuse=True)
            nsi = nop.ins.sync_info
            if nsi is None:
                nop.ins.sync_info = type(si)(on_wait=[w], on_update=[])
            else:
                nsi.on_wait.append(w)
    self.nc.sync.drain()
    self.nc.all_engine_barrier()
    assert self.sems is not None
    popped = self.nc._tile_sem_poison_stack.pop()
    assert popped is self._sem_poison
    self.nc.clear_and_free_semaphores(list(self.sems.allocated().values()))
    self.nc.all_engine_barrier()


tile.TileContext._drain_and_barrier = _patched_drain_and_barrier


def _split_sync_waits(nc, max_waits=1):
    """walrus in this container allows only one sem-wait per instruction:
    hoist extra waits onto same-engine NOPs spliced immediately before."""
    for fn in nc.m.functions:
        for bb in fn.blocks:
            out = []
            for inst in bb.instructions:
                si = inst.sync_info
                if si is not None and len(si.on_wait) > max_waits:
                    extra = list(si.on_wait[max_waits:])
                    del si.on_wait[max_waits:]
                    for w in extra:
                        out.append(mybir.InstNoOp(
                            name=nc.get_next_instruction_name(),
                            engine=inst.engine,
                            sync_info=mybir.SyncInfo(on_wait=[w], on_update=[]),
                            bass_nofuse=True,
                        ))
                out.append(inst)
            bb.instructions = out


# ---------------------------------------------------------------------------
# Device kernel
# ---------------------------------------------------------------------------
def _build(ncores=M, split_waits=True, phases=('p1', 'p2', 'p3'), mock_cc=False, debug=False):
    nc = bass.Bass()
    p_g = nc.declare_dram_parameter("g_rows", [128, NG * CPG * 128], BF16, isOutput=False)
    p_ablk = nc.declare_dram_parameter("a_blk", [NG * 128, CPG * 128], BF16, isOutput=False)
    p_ssz = nc.declare_dram_parameter("s_sizes", [1, NG * CPG * 128], BF16, isOutput=False)
    p_wcT = nc.declare_dram_parameter("wcT", [E, D], BF16, isOutput=False)
    p_wcb = nc.declare_dram_parameter("wcb", [1, D], BF16, isOutput=False)
    p_wihT = {d: nc.declare_dram_parameter(f"wihT_{d}", [D, 3 * H], BF16, isOutput=False)
              for d in ("f", "b")}
    p_xbias = {d: nc.declare_dram_parameter(f"xbias_{d}", [H, 3], F32, isOutput=False)
               for d in ("f", "b")}
    p_whhT = {d: nc.declare_dram_parameter(f"whhT_{d}", [H + 1, 3 * H], F32, isOutput=False)
              for d in ("f", "b")}
    p_lblT = nc.declare_dram_parameter("lblT", [H + 1, 2 * C], F32, isOutput=False)
    p_sinit = nc.declare_dram_parameter("slab_init", [H + 1, 8 * 32], F32, isOutput=False)
    p_out = nc.declare_dram_parameter("out", [BL, C], F32, isOutput=True)
    p_dbg = nc.declare_dram_parameter("dbg", [128, 16 * 64], F32, isOutput=True) if debug else None

    with tile.TileContext(nc) as tc:
        with tc.tile_pool(name="const", bufs=1) as const:
            wcT_sb = const.tile([E, D], BF16)
            nc.sync.dma_start(wcT_sb[:], p_wcT[:])
            wcb_sb = const.tile([1, D], BF16)
            nc.sync.dma_start(wcb_sb[:], p_wcb[:])
            ssz_sb = const.tile([1, NG * CPG * 128], BF16)
            nc.sync.dma_start(ssz_sb[:], p_ssz[:])
            whhT_sb = {}
            wihT_sb = {}
            xbias_sb = {}
            for d in ("f", "b"):
                whhT_sb[d] = const.tile([H + 1, 3 * H], F32, name=f"whhT{d}")
                nc.sync.dma_start(whhT_sb[d][:], p_whhT[d][:])
                wihT_sb[d] = const.tile([D, 3 * H], BF16, name=f"wihT{d}")
                nc.sync.dma_start(wihT_sb[d][:], p_wihT[d][:])
                xbias_sb[d] = const.tile([H, 3], F32, name=f"xbias{d}")
                nc.sync.dma_start(xbias_sb[d][:], p_xbias[d][:])
            lblT_sb = const.tile([H + 1, 2 * C], F32)
            nc.sync.dma_start(lblT_sb[:], p_lblT[:])

            enc_sb = const.tile([128, T], BF16)
            # xW slabs: [H, dir, gate, b, l] for r/z ; [H, dir, b, l] for n
            xw_rz = const.tile([H, 2 * 2 * BL * L], BF16)
            xw_n = const.tile([H, 2 * BL * L], BF16)
            identB = const.tile([128, 128], BF16)
            make_identity(nc, identB[:])

            # ---------------- P1: tree-aggregate + project + enc -----------
            with tc.tile_pool(name="p1", bufs=2) as p1, \
                 tc.tile_pool(name="p1ps", bufs=2, space="PSUM") as p1ps:
                for g in range(NG if "p1" in phases else 0):
                    g_sb = p1.tile([128, CPG * 128], BF16, tag="g")
                    nc.sync.dma_start(g_sb[:], p_g[:, g * CPG * 128:(g + 1) * CPG * 128])
                    g_v = g_sb[:].rearrange("p (c e) -> p c e", c=CPG)
                    ab_sb = p1.tile([128, CPG * 128], BF16, tag="ab")
                    nc.sync.dma_start(ab_sb[:], p_ablk[g * 128:(g + 1) * 128, :])
                    for k in range(CPG // 4):
                        hT_ps = p1ps.tile([128, 512], F32, tag="hT")
                        for q in range(4):
                            c = k * 4 + q
                            tmp_ps = p1ps.tile([128, 128], F32, tag="tmp", bufs=4)
                            nc.tensor.matmul(
                                out=tmp_ps[:], lhsT=g_v[:, c, :],
                                rhs=ab_sb[:, c * 128:(c + 1) * 128],
                                start=True, stop=True)
                            tmp_sb = p1.tile([128, 128], BF16, tag="tmp_sb", bufs=4)
                            nc.scalar.copy(out=tmp_sb[:], in_=tmp_ps[:])
                            # subtree-size bias first (rank-1, no dep on tmp)
                            base = g * CPG * 128 + c * 128
                            nc.tensor.matmul(
                                out=hT_ps[:, q * 128:(q + 1) * 128],
                                lhsT=wcb_sb[:],
                                rhs=ssz_sb[0:1, base:base + 128],
                                start=True, stop=False)
                            nc.tensor.matmul(
                                out=hT_ps[:, q * 128:(q + 1) * 128],
                                lhsT=wcT_sb[:], rhs=tmp_sb[:],
                                start=False, stop=True)
                        blk = g * (CPG // 4) + k   # 32 statements per block
                        nc.vector.tensor_reduce(
                            out=enc_sb[:, blk * 32:(blk + 1) * 32],
                            in_=hT_ps[:].rearrange("p (s x) -> p s x", x=N),
                            axis=AX.X, op=OP.max,
                        )
            if "p1" in phases:
                nc.scalar.activation(enc_sb[:], enc_sb[:], ACTF.Relu)

            # ---------------- P2: xW = enc @ Wih^T + bias -------------------
            with tc.tile_pool(name="p2ps", bufs=2, space="PSUM") as p2ps:
                for di, d in enumerate(("f", "b")) if "p2" in phases else ():
                    for gi in range(3):
                        for tch in range(4):
                            ps = p2ps.tile([H, 512], F32, tag="xw")
                            nc.tensor.matmul(
                                out=ps[:],
                                lhsT=wihT_sb[d][:, gi * H:(gi + 1) * H],
                                rhs=enc_sb[:, tch * 512:(tch + 1) * 512],
                                start=True, stop=True,
                            )
                            if gi < 2:
                                dest = xw_rz[:].rearrange(
                                    "p (g d b l) -> p g d b l", d=2, g=2, b=BL)[
                                    :, gi, di, tch * 4:(tch + 1) * 4, :]
                            else:
                                dest = xw_n[:].rearrange(
                                    "p (d b l) -> p d b l", d=2, b=BL)[
                                    :, di, tch * 4:(tch + 1) * 4, :]
                            nc.scalar.activation(dest, ps[:], ACTF.Identity,
                                                 bias=xbias_sb[d][:, gi:gi + 1])

            # ---------------- P3: bidirectional GRU scan --------------------
            slab = const.tile([H + 1, 8 * 32], F32)       # [H+1, slot, 2*BL]
            slab_v = slab[:].rearrange("q (s b) -> q s b", s=8)
            nc.sync.dma_start(slab[:], p_sinit[:])        # zeros + ones bias row
            pool_t = const.tile([H, 32], F32)
            from dataclasses import replace as _rep
            xwrz_base = xw_rz[:]
            xwn_base = xw_n[:]

            def xwrz_step(i):
                # element (g, dir, b): fwd at l=i, bwd at l=127-i
                sd = BL * L + (L - 1) - 2 * i
                return _rep(xwrz_base, offset=xwrz_base.offset + i,
                            ap=type(xwrz_base.ap)(
                                [list(xwrz_base.ap[0]), [2 * BL * L, 2], [sd, 2], [L, BL]]))

            def xwn_step(i):
                sd = BL * L + (L - 1) - 2 * i
                return _rep(xwn_base, offset=xwn_base.offset + i,
                            ap=type(xwn_base.ap)(
                                [list(xwn_base.ap[0]), [sd, 2], [L, BL]]))

            with tc.tile_pool(name="p3", bufs=4) as p3, \
                 tc.tile_pool(name="p3ps", bufs=2, space="PSUM") as p3ps:
                # h' split as v + zh: v = 2q*zbar (chain tail), zh = (1-zbar)h'
                # (computed early on Pool). PE consumes both via separate
                # accumulating matmuls, so hnew is never materialized on-chain.
                zhs = [const.tile([H + 1, 32], F32, name=f"zh{j}") for j in range(2)]
                v_init = const.tile([H, 32], F32)
                if "p3" in phases:
                    for j in range(2):
                        nc.sync.dma_start(zhs[j][:], p_sinit[:, 0:32])  # ones
                    nc.vector.memset(v_init[:], 0.0)
                for i in range(L if "p3" in phases else 0):
                    s, pv = i % 8, (i - 1) % 8
                    zh_rd = zhs[(i - 1) % 2]
                    zh_wr = zhs[i % 2]
                    vprev = v_init[:] if i == 0 else v_last
                    ps_rz = p3ps.tile([H, 64], F32, tag="rz", bufs=4)
                    ps_n = p3ps.tile([H, 32], F32, tag="n", bufs=3)
                    # cols: [r_f r_b | -z_f -z_b] (z gate negated host-side).
                    # xW + zh-part matmuls land first (no dep on the chain
                    # tail v); v-part matmuls accumulate when v arrives.
                    # accumulation groups must stay contiguous in program
                    # order (interleaved start/stop corrupts PE state): all
                    # ps_rz matmuls first, then the two ps_n groups.
                    nc.tensor.matmul(out=ps_rz[:], lhsT=identB[0:H, 0:H],
                                     rhs=xwrz_step(i), start=True, stop=False,
                                     skip_group_check=True)
                    for di, d in enumerate(("f", "b")):
                        bc = slice(di * 16, di * 16 + 16)
                        nc.tensor.matmul(out=ps_rz[:, di * 16:di * 16 + 16],
                                         lhsT=whhT_sb[d][0:H, 0:H],
                                         rhs=zh_rd[0:H, bc], start=False, stop=False,
                                         skip_group_check=True)
                        nc.tensor.matmul(out=ps_rz[:, 32 + di * 16:32 + di * 16 + 16],
                                         lhsT=whhT_sb[d][0:H, H:2 * H],
                                         rhs=zh_rd[0:H, bc], start=False, stop=False,
                                         skip_group_check=True)
                    for di, d in enumerate(("f", "b")):
                        bc = slice(di * 16, di * 16 + 16)
                        nc.tensor.matmul(out=ps_rz[:, di * 16:di * 16 + 16],
                                         lhsT=whhT_sb[d][0:H, 0:H],
                                         rhs=vprev[0:H, bc], start=False,
                                         stop=False, skip_group_check=True)
                        nc.tensor.matmul(out=ps_rz[:, 32 + di * 16:32 + di * 16 + 16],
                                         lhsT=whhT_sb[d][0:H, H:2 * H],
                                         rhs=vprev[0:H, bc], start=False,
                                         stop=(di == 1), skip_group_check=True)
                    for di, d in enumerate(("f", "b")):
                        bc = slice(di * 16, di * 16 + 16)
                        nc.tensor.matmul(out=ps_n[:, di * 16:di * 16 + 16],
                                         lhsT=whhT_sb[d][:, 2 * H:3 * H],
                                         rhs=zh_rd[:, bc], start=True, stop=False,
                                         skip_group_check=True)
                        nc.tensor.matmul(out=ps_n[:, di * 16:di * 16 + 16],
                                         lhsT=whhT_sb[d][0:H, 2 * H:3 * H],
                                         rhs=vprev[0:H, bc], start=False,
                                         stop=True, skip_group_check=True)
                    r_sb = p3.tile([H, 32], F32, tag="r_sb")
                    nc.scalar.activation(r_sb[:], ps_rz[:, 0:32], ACTF.Sigmoid)
                    zbar = p3.tile([H, 32], F32, tag="zbar")
                    nc.scalar.activation(zbar[:], ps_rz[:, 32:64], ACTF.Sigmoid)
                    # off-chain (Pool): zbar2, zh, and the pooled state h'
                    zbar2 = p3.tile([H, 32], F32, tag="zbar2")
                    nc.gpsimd.tensor_tensor(out=zbar2[:], in0=zbar[:], in1=zbar[:],
                                            op=OP.add)
                    nc.gpsimd.tensor_tensor(out=zh_wr[0:H, :], in0=zbar[:],
                                            in1=slab_v[0:H, pv, :], op=OP.mult)
                    nc.gpsimd.tensor_tensor(out=zh_wr[0:H, :], in0=slab_v[0:H, pv, :],
                                            in1=zh_wr[0:H, :], op=OP.subtract)
                    u = p3.tile([H, 32], F32, tag="u")
                    nc.vector.tensor_tensor(out=u[:], in0=r_sb[:],
                                            in1=ps_n[:], op=OP.mult)
                    t2 = p3.tile([H, 32], F32, tag="t2")
                    nc.vector.tensor_tensor(out=t2[:], in0=u[:],
                                            in1=xwn_step(i), op=OP.add)
                    # tanh(t2) = 2*sigmoid(2*t2) - 1; state kept offset by +1
                    # (h' = h+1; all bias corrections folded host-side)
                    q = p3.tile([H, 32], F32, tag="q")
                    nc.scalar.activation(q[:], t2[:], ACTF.Sigmoid, scale=2.0)
                    v = p3.tile([H, 32], F32, tag="v")
                    nc.vector.tensor_tensor(out=v[:], in0=q[:], in1=zbar2[:],
                                            op=OP.mult)
                    v_last = v[:]
                    # pooled state for max-pool / next zh (Pool, off-chain)
                    nc.gpsimd.tensor_tensor(out=slab_v[0:H, s, :], in0=v[:],
                                            in1=zh_wr[0:H, :], op=OP.add)
                    if debug and i < 8:
                        nc.sync.dma_start(p_dbg[0:H, i * 128:i * 128 + 32], slab_v[0:H, s, :])
                        nc.sync.dma_start(p_dbg[0:H, i * 128 + 32:i * 128 + 64], v[:])
                        psn_sb = p3.tile([H, 32], F32, tag="dbg_psn")
                        nc.vector.tensor_copy(out=psn_sb[:], in_=ps_n[:])
                        nc.sync.dma_start(p_dbg[0:H, i * 128 + 64:i * 128 + 96], psn_sb[:])
                        nc.sync.dma_start(p_dbg[0:H, i * 128 + 96:i * 128 + 128], r_sb[:])
                    if i % 8 == 7:
                        if i == 7:
                            nc.vector.tensor_reduce(
                                out=pool_t[:], in_=slab[0:H, :].rearrange("q (s b) -> q b s", s=8),
                                axis=AX.X, op=OP.max)
                        else:
                            red = p3.tile([H, 32], F32, tag="red")
                            nc.vector.tensor_reduce(
                                out=red[:], in_=slab[0:H, :].rearrange("q (s b) -> q b s", s=8),
                                axis=AX.X, op=OP.max)
                            nc.vector.tensor_tensor(out=pool_t[:], in0=pool_t[:],
                                                    in1=red[:], op=OP.max)

                # ------------ classifier ---------------------------------
                if "p3" in phases:
                    pe = p3.tile([H + 1, 32], F32, tag="pe")
                    nc.sync.dma_start(pe[H:H + 1, :], p_sinit[H:H + 1, 0:32])
                    nc.vector.tensor_copy(out=pe[0:H, :], in_=pool_t[:])
                    ps_o = p3ps.tile([BL, C], F32, tag="out", bufs=1)
                    nc.tensor.matmul(out=ps_o[:], lhsT=pe[:, 0:16], rhs=lblT_sb[:, 0:C],
                                     start=True, stop=False)
                    nc.tensor.matmul(out=ps_o[:], lhsT=pe[:, 16:32], rhs=lblT_sb[:, C:2 * C],
                                     start=False, stop=True)
                    out_sb = p3.tile([BL, C], F32, tag="out_sb")
                    nc.vector.tensor_copy(out=out_sb[:], in_=ps_o[:])
                    nc.sync.dma_start(p_out[:], out_sb[:])
    lower_extended_insts(nc)
    if split_waits:
        _split_sync_waits(nc)
    return nc


_NC_CACHE = None


def _get_nc():
    global _NC_CACHE
    if _NC_CACHE is None:
        _NC_CACHE = _build()
    return _NC_CACHE


# ---------------------------------------------------------------------------
# Host-side input prep (sharding + index/layout preprocessing only)
# ---------------------------------------------------------------------------
def _ancestor_blocks(parents_core: np.ndarray):
    """parents_core [T, N] -> (block-diagonal ancestor-closure rhs tiles,
    subtree sizes).

    A[t, p, j] = 1 iff p is an ancestor-or-self of j; the device applies the
    tree scatter-add as H^T = G_chunk^T @ A_blk on PE.
    Returns ([NG*128, CPG*128] bf16, [NG, CPG*128] bf16 subtree sizes).
    """
    par = parents_core
    A = np.zeros((T, N, N), np.float32)
    rng = np.arange(N)
    A[:, rng, rng] = 1.0
    tidx = np.arange(T)
    for j in range(1, N):
        A[:, :, j] += A[tidx, :, par[:, j]]
    s = A.sum(axis=2)                # [T, N] subtree sizes
    # rhs_blk[(s,j),(s,p)] = A[stmt, p, j]
    At = np.transpose(A, (0, 2, 1))  # [T, j, p]
    blk = np.zeros((NCH, 128, 128), np.float32)
    Ar = At.reshape(NCH, 8, N, N)
    for st in range(8):
        blk[:, st * N:(st + 1) * N, st * N:(st + 1) * N] = Ar[:, st]
    # group-pack: [NG, 128(j-row), CPG, 128(p-col)] -> [NG*128, CPG*128]
    grp = blk.reshape(NG, CPG, 128, 128).transpose(0, 2, 1, 3).reshape(NG * 128, CPG * 128)
    ssz = s.reshape(1, NG * CPG * 128)
    return grp.astype(ml_dtypes.bfloat16), ssz.astype(ml_dtypes.bfloat16)


def kernel(tokens, parents, emb, Wc_w, Wc_b,
           Wih_f, Whh_f, bih_f, bhh_f,
           Wih_b, Whh_b, bih_b, bhh_b,
           lbl_w, lbl_b):
    tokens = np.asarray(tokens)
    parents = np.asarray(parents)
    bf = ml_dtypes.bfloat16
    emb_bf = np.asarray(emb, np.float32).astype(bf)           # [V, E]

    wcT = Wc_w.T.astype(bf)                                   # [E, D]
    wcb = np.asarray(Wc_b, np.float32).astype(bf).reshape(1, D)

    def pack_dir(Wih, Whh, bih, bhh):
        wihT = np.asarray(Wih, np.float32).T.astype(bf)       # [D, 3H]
        Whh = np.asarray(Whh, np.float32)
        bih = np.asarray(bih, np.float32)
        bhh = np.asarray(bhh, np.float32)
        rs = Whh.reshape(3, H, H).sum(axis=2)   # rowsums per gate (h'=h+1 fold)
        whhT = np.zeros((H + 1, 3 * H), np.float32)
        whhT[:H, :] = Whh.T
        whhT[:H, H:2 * H] *= -1.0               # z gate negated: sigmoid(-a_z)=zbar
        whhT[H, 2 * H:3 * H] = bhh[2 * H:3 * H] - rs[2]
        wihT[:, H:2 * H] *= -1.0
        xbias = np.stack([
            bih[0:H] + bhh[0:H] - rs[0],
            -(bih[H:2 * H] + bhh[H:2 * H] - rs[1]),
            bih[2 * H:3 * H],
        ], axis=1)                                            # [H, 3]
        return wihT, whhT, xbias

    wihT_f, whhT_f, xbias_f = pack_dir(Wih_f, Whh_f, bih_f, bhh_f)
    wihT_b, whhT_b, xbias_b = pack_dir(Wih_b, Whh_b, bih_b, bhh_b)

    lblT = np.zeros((H + 1, 2 * C), np.float32)
    lblT[:H, 0:C] = np.asarray(lbl_w, np.float32)[:, 0:H].T
    lblT[H, 0:C] = (np.asarray(lbl_b, np.float32)
                    - np.asarray(lbl_w, np.float32).sum(axis=1))
    lblT[:H, C:2 * C] = np.asarray(lbl_w, np.float32)[:, H:2 * H].T

    slab_init = np.ones((H + 1, 8 * 32), np.float32)  # h' = h+1 -> h0' = 1

    in_maps = []
    for i in range(M):
        bs = slice(i * BL, (i + 1) * BL)
        tok = tokens[bs].reshape(-1)                          # [T*N] b-major
        par = parents[bs].reshape(T, N)
        a_blk, ssz = _ancestor_blocks(par)
        # g_rows[p, ((g c) e)] = emb_bf[tok[g*4096 + c*128 + p]]
        idx = tok.reshape(NG * CPG, 128).T                    # [128, NG*CPG]
        g_rows = np.ascontiguousarray(emb_bf[idx]).reshape(128, NG * CPG * 128)
        in_maps.append(dict(
            g_rows=g_rows,
            a_blk=a_blk, s_sizes=ssz,
            wcT=wcT, wcb=wcb,
            wihT_f=wihT_f, wihT_b=wihT_b, slab_init=slab_init,
            xbias_f=xbias_f, xbias_b=xbias_b,
            whhT_f=whhT_f, whhT_b=whhT_b,
            lblT=lblT,
        ))

    nc = _get_nc()
    res = run_bass_kernel_spmd(nc, in_maps, core_ids=list(range(M)))
    return np.concatenate([res.results[i]["out"] for i in range(M)], axis=0)
